# revision 8
# baseline (speedup 1.0000x reference)
"""Trainium2 kernel for Bernoulli.sample(unique=True) dedup pipeline.

Pipeline (matches the jax reference bit-exactly, with x64 disabled):
  probs = sigmoid(2*theta)                    [host, jax CPU backend]
  bits  = (u < probs)                         [device: DVE is_lt, 8 cores]
  key   = int32 packing of bits (low 32 only; shifts >=32 give 0,
          shift 31 wraps the sign bit)        [host, numpy]
  jnp.unique(keys, size=N, fill=-1) + scatter bits rows (last write wins)
                                              [host, numpy replication]

Device work is the memory-bound part: reading u (192 MB) and writing the
0/1 bit tensor (48 MB), data-parallel over samples across 8 NeuronCores.
"""

import os
import numpy as np

NUM_BITS = 48
N_TOTAL = 1_000_000
N_CORES = 8

# Tiling: 128 partitions x C sample-chunks x 48 bits per tile. The
# schedule (list of C per tile) was picked by sweeping the production
# cost model (TimelineSim); 12x78+44 beat uniform 98x10 by ~3 us.
SCHEDULE = [78] * 12 + [44]
N_PER_CORE = 128 * sum(SCHEDULE)  # 125440
N_PAD = N_PER_CORE * N_CORES      # 1003520 (input padded up to this)

_CACHE = {}


def _emit_body(tc, u_ap, p_ap, b_ap, schedule, repeat=1):
    """Emit the per-core program: for each tile (128 partitions x C samples
    x 48 bits), DMA u in, compare against the broadcast probs tile on DVE,
    DMA the uint8 bits out. Row i of the bits output corresponds to row i
    of u.

    repeat > 1 re-runs the whole pass (same input/output regions) — only
    used for benchmarking kernel duration via the t(repeat) slope."""
    import concourse.mybir as mybir
    from concourse.alu_op_type import AluOpType

    nc = tc.nc
    cmax = max(schedule)

    with (
        tc.tile_pool(name="pin", bufs=3) as pin,
        tc.tile_pool(name="pout", bufs=3) as pout,
        tc.tile_pool(name="pprobs", bufs=1) as pp,
    ):
        pt = pp.tile([128, NUM_BITS], mybir.dt.float32)
        nc.sync.dma_start(pt[:, :], p_ap)

        for _ in range(repeat):
            off = 0
            for c in schedule:
                uv = u_ap[off * 128:(off + c) * 128, :].rearrange(
                    "(p c) b -> p c b", p=128)
                bv = b_ap[off * 128:(off + c) * 128, :].rearrange(
                    "(p c) b -> p c b", p=128)

                ut = pin.tile([128, cmax * NUM_BITS], mybir.dt.float32,
                              tag="in")
                u3 = ut[:, :c * NUM_BITS].rearrange("p (c b) -> p c b",
                                                    b=NUM_BITS)
                nc.sync.dma_start(u3, uv)

                bt = pout.tile([128, cmax * NUM_BITS], mybir.dt.uint8,
                               tag="out")
                b3 = bt[:, :c * NUM_BITS].rearrange("p (c b) -> p c b",
                                                    b=NUM_BITS)
                pb = pt[:, :].unsqueeze(1).to_broadcast((128, c, NUM_BITS))
                nc.vector.tensor_tensor(b3, u3, pb, AluOpType.is_lt)

                nc.sync.dma_start(bv, b3)
                off += c


def _build(n_per_core, schedule, repeat=1):
    import concourse.bacc as bacc
    import concourse.mybir as mybir
    import concourse.tile as tile

    assert n_per_core == 128 * sum(schedule)
    nc = bacc.Bacc("TRN2", target_bir_lowering=False, debug=False)
    u = nc.dram_tensor("u", [n_per_core, NUM_BITS], mybir.dt.float32,
                       kind="ExternalInput")
    p = nc.dram_tensor("p", [128, NUM_BITS], mybir.dt.float32,
                       kind="ExternalInput")
    b = nc.dram_tensor("bits", [n_per_core, NUM_BITS], mybir.dt.uint8,
                       kind="ExternalOutput")
    with tile.TileContext(nc) as tc:
        _emit_body(tc, u.ap(), p.ap()[:, :], b.ap(), schedule, repeat)
    nc.finalize()
    return nc


def _probs_f32(theta):
    """sigmoid(2*theta) computed exactly as the reference does, on the jax
    CPU backend (bit-exact vs a CPU-run reference)."""
    import jax

    cpu = jax.local_devices(backend="cpu")[0]
    theta_cpu = jax.device_put(np.asarray(theta, np.float32), cpu)
    with jax.default_device(cpu):
        probs = jax.nn.sigmoid(2.0 * theta_cpu)
    return np.asarray(probs, np.float32)


def _run_device(u_np, probs):
    from concourse.bass_utils import run_bass_kernel_spmd

    if "nc" not in _CACHE:
        _CACHE["nc"] = _build(N_PER_CORE, SCHEDULE)
    nc = _CACHE["nc"]

    # Pad with 2.0 (> any sigmoid output) => padding rows get all-zero bits.
    u_pad = np.empty((N_PAD, NUM_BITS), np.float32)
    u_pad[:N_TOTAL] = u_np
    u_pad[N_TOTAL:] = 2.0

    p_tile = np.broadcast_to(probs, (128, NUM_BITS)).copy()
    in_maps = [
        {"u": u_pad[k * N_PER_CORE:(k + 1) * N_PER_CORE], "p": p_tile}
        for k in range(N_CORES)
    ]
    res = run_bass_kernel_spmd(nc, in_maps, core_ids=list(range(N_CORES)))
    _CACHE["last_result"] = res
    bits = np.concatenate([r["bits"] for r in res.results])[:N_TOTAL]
    return bits


def _dedup(bits_u8):
    """Replicate jnp.unique(keys, size=N, fill_value=-1) + scatter, where
    keys are the int32-overflowed packing (low 32 bits only) and duplicate
    scatter indices resolve last-write-wins (XLA CPU behavior)."""
    n = bits_u8.shape[0]
    packed = np.packbits(bits_u8, axis=1, bitorder="little")      # [n, 6]
    key32 = packed[:, :4].copy().view(np.uint32).ravel().view(np.int32)

    order = np.argsort(key32, kind="stable")
    sk = key32[order]
    mask = np.empty(n, bool)
    mask[0] = True
    mask[1:] = sk[1:] != sk[:-1]
    starts = np.nonzero(mask)[0]
    nuniq = len(starts)

    counts = np.zeros(n, np.int32)
    counts[:nuniq] = np.diff(np.append(starts, n)).astype(np.int32)

    winners = np.maximum.reduceat(order, starts)                  # last index
    unique_bits = np.zeros((n, NUM_BITS), np.int8)
    unique_bits[:nuniq] = bits_u8[winners]
    return unique_bits, counts


def kernel(theta, u):
    theta = np.asarray(theta, np.float32)
    u = np.ascontiguousarray(np.asarray(u, np.float32))
    assert u.shape == (N_TOTAL, NUM_BITS)

    probs = _probs_f32(theta)
    bits = _run_device(u, probs)
    return _dedup(bits)


# revision 12
# speedup vs baseline: 1.3701x; 1.3701x over previous
"""Trainium2 kernel for Bernoulli.sample(unique=True) dedup pipeline.

Pipeline (matches the jax reference bit-exactly, with x64 disabled):
  probs = sigmoid(2*theta)                    [host, jax CPU backend]
  bits  = (u < probs)                         [device: DVE is_lt, 8 cores]
  key   = int32 packing of bits (low 32 only; shifts >=32 give 0,
          shift 31 wraps the sign bit)        [host, numpy]
  jnp.unique(keys, size=N, fill=-1) + scatter bits rows (last write wins)
                                              [host, numpy replication]

Device work is the memory-bound part: reading u (192 MB) and writing the
0/1 bit tensor (48 MB), data-parallel over samples across 8 NeuronCores.
"""

import os
import numpy as np

NUM_BITS = 48
N_TOTAL = 1_000_000
N_CORES = 8

# Tiling: 128 partitions x C sample-chunks x 48 bits per tile. The
# schedule (list of C per tile) was picked by sweeping the production
# cost model (TimelineSim); 12x78+44 beat uniform 98x10 by ~3 us.
SCHEDULE = [78] * 12 + [44]
N_PER_CORE = 128 * sum(SCHEDULE)  # 125440
N_PAD = N_PER_CORE * N_CORES      # 1003520 (input padded up to this)

# Pack bit pairs on-device (pair[k] = bits[k] + 2*bits[k+24], one extra
# DVE scalar_tensor_tensor per tile) to halve the output DMA bytes.
PACK_PAIRS = True
NUM_PAIRS = NUM_BITS // 2

_CACHE = {}


def _emit_body(tc, u_ap, p_ap, b_ap, schedule, repeat=1):
    """Emit the per-core program: for each tile (128 partitions x C samples
    x 48 bits), DMA u in, compare against the broadcast probs tile on DVE,
    DMA the uint8 bits out. Row i of the bits output corresponds to row i
    of u.

    repeat > 1 re-runs the whole pass (same input/output regions) — only
    used for benchmarking kernel duration via the t(repeat) slope."""
    import concourse.mybir as mybir
    from concourse.alu_op_type import AluOpType

    nc = tc.nc
    cmax = max(schedule)
    pack = PACK_PAIRS

    with (
        tc.tile_pool(name="pin", bufs=3) as pin,
        tc.tile_pool(name="pbits", bufs=3) as pbits,
        tc.tile_pool(name="pout", bufs=3) as pout,
        tc.tile_pool(name="pprobs", bufs=1) as pp,
    ):
        pt = pp.tile([128, NUM_BITS], mybir.dt.float32)
        nc.sync.dma_start(pt[:, :], p_ap)

        for _ in range(repeat):
            off = 0
            for c in schedule:
                uv = u_ap[off * 128:(off + c) * 128, :].rearrange(
                    "(p c) b -> p c b", p=128)
                bv = b_ap[off * 128:(off + c) * 128, :].rearrange(
                    "(p c) b -> p c b", p=128)

                ut = pin.tile([128, cmax * NUM_BITS], mybir.dt.float32,
                              tag="in")
                u3 = ut[:, :c * NUM_BITS].rearrange("p (c b) -> p c b",
                                                    b=NUM_BITS)
                nc.sync.dma_start(u3, uv)

                bt = pbits.tile([128, cmax * NUM_BITS], mybir.dt.uint8,
                                tag="bits")
                b3 = bt[:, :c * NUM_BITS].rearrange("p (c b) -> p c b",
                                                    b=NUM_BITS)
                pb = pt[:, :].unsqueeze(1).to_broadcast((128, c, NUM_BITS))
                nc.vector.tensor_tensor(b3, u3, pb, AluOpType.is_lt)

                if pack:
                    # pair[k] = bits[k] + 2*bits[k+24], k in [0, 24)
                    qt = pout.tile([128, cmax * NUM_PAIRS], mybir.dt.uint8,
                                   tag="out")
                    q3 = qt[:, :c * NUM_PAIRS].rearrange(
                        "p (c k) -> p c k", k=NUM_PAIRS)
                    lo = b3[:, :, 0:NUM_PAIRS]
                    hi = b3[:, :, NUM_PAIRS:NUM_BITS]
                    nc.vector.scalar_tensor_tensor(
                        q3, hi, 2.0, lo, AluOpType.mult, AluOpType.add)
                    nc.sync.dma_start(bv, q3)
                else:
                    nc.sync.dma_start(bv, b3)
                off += c


def _build(n_per_core, schedule, repeat=1):
    import concourse.bacc as bacc
    import concourse.mybir as mybir
    import concourse.tile as tile

    assert n_per_core == 128 * sum(schedule)
    nc = bacc.Bacc("TRN2", target_bir_lowering=False, debug=False)
    u = nc.dram_tensor("u", [n_per_core, NUM_BITS], mybir.dt.float32,
                       kind="ExternalInput")
    p = nc.dram_tensor("p", [128, NUM_BITS], mybir.dt.float32,
                       kind="ExternalInput")
    out_w = NUM_PAIRS if PACK_PAIRS else NUM_BITS
    b = nc.dram_tensor("bits", [n_per_core, out_w], mybir.dt.uint8,
                       kind="ExternalOutput")
    with tile.TileContext(nc) as tc:
        _emit_body(tc, u.ap(), p.ap()[:, :], b.ap(), schedule, repeat)
    nc.finalize()
    return nc


def _probs_f32(theta):
    """sigmoid(2*theta) computed exactly as the reference does, on the jax
    CPU backend (bit-exact vs a CPU-run reference)."""
    import jax

    cpu = jax.local_devices(backend="cpu")[0]
    theta_cpu = jax.device_put(np.asarray(theta, np.float32), cpu)
    with jax.default_device(cpu):
        probs = jax.nn.sigmoid(2.0 * theta_cpu)
    return np.asarray(probs, np.float32)


def _run_device(u_np, probs):
    from concourse.bass_utils import run_bass_kernel_spmd

    if "nc" not in _CACHE:
        _CACHE["nc"] = _build(N_PER_CORE, SCHEDULE)
    nc = _CACHE["nc"]

    # Pad with 2.0 (> any sigmoid output) => padding rows get all-zero bits.
    u_pad = np.empty((N_PAD, NUM_BITS), np.float32)
    u_pad[:N_TOTAL] = u_np
    u_pad[N_TOTAL:] = 2.0

    p_tile = np.broadcast_to(probs, (128, NUM_BITS)).copy()
    in_maps = [
        {"u": u_pad[k * N_PER_CORE:(k + 1) * N_PER_CORE], "p": p_tile}
        for k in range(N_CORES)
    ]
    res = run_bass_kernel_spmd(nc, in_maps, core_ids=list(range(N_CORES)))
    _CACHE["last_result"] = res
    out = np.concatenate([r["bits"] for r in res.results])[:N_TOTAL]
    if PACK_PAIRS:
        bits = np.empty((N_TOTAL, NUM_BITS), np.uint8)
        bits[:, :NUM_PAIRS] = out & 1
        bits[:, NUM_PAIRS:] = out >> 1
    else:
        bits = out
    return bits


def _dedup(bits_u8):
    """Replicate jnp.unique(keys, size=N, fill_value=-1) + scatter, where
    keys are the int32-overflowed packing (low 32 bits only) and duplicate
    scatter indices resolve last-write-wins (XLA CPU behavior)."""
    n = bits_u8.shape[0]
    packed = np.packbits(bits_u8, axis=1, bitorder="little")      # [n, 6]
    key32 = packed[:, :4].copy().view(np.uint32).ravel().view(np.int32)

    order = np.argsort(key32, kind="stable")
    sk = key32[order]
    mask = np.empty(n, bool)
    mask[0] = True
    mask[1:] = sk[1:] != sk[:-1]
    starts = np.nonzero(mask)[0]
    nuniq = len(starts)

    counts = np.zeros(n, np.int32)
    counts[:nuniq] = np.diff(np.append(starts, n)).astype(np.int32)

    winners = np.maximum.reduceat(order, starts)                  # last index
    unique_bits = np.zeros((n, NUM_BITS), np.int8)
    unique_bits[:nuniq] = bits_u8[winners]
    return unique_bits, counts


def kernel(theta, u):
    theta = np.asarray(theta, np.float32)
    u = np.ascontiguousarray(np.asarray(u, np.float32))
    assert u.shape == (N_TOTAL, NUM_BITS)

    probs = _probs_f32(theta)
    bits = _run_device(u, probs)
    return _dedup(bits)


# revision 13
# speedup vs baseline: 1.6488x; 1.2034x over previous
"""Trainium2 kernel for Bernoulli.sample(unique=True) dedup pipeline.

Pipeline (matches the jax reference bit-exactly, with x64 disabled):
  probs = sigmoid(2*theta)                    [host, jax CPU backend]
  bits  = (u < probs)                         [device: DVE is_lt, 8 cores]
  key   = int32 packing of bits (low 32 only; shifts >=32 give 0,
          shift 31 wraps the sign bit)        [host, numpy]
  jnp.unique(keys, size=N, fill=-1) + scatter bits rows (last write wins)
                                              [host, numpy replication]

Device work is the memory-bound part: reading u (192 MB) and writing the
0/1 bit tensor (48 MB), data-parallel over samples across 8 NeuronCores.
"""

import os
import numpy as np

NUM_BITS = 48
N_TOTAL = 1_000_000
N_CORES = 8

# Tiling: 128 partitions x C sample-chunks x 48 bits per tile. The
# schedule (list of C per tile) was picked by head-to-head HW timing:
# large 3.4 MB in-DMA transfers beat the cost model's smaller-tile
# preference (interleaved slope benchmark, ordering stable in-run).
SCHEDULE = [140] * 7
N_PER_CORE = 128 * sum(SCHEDULE)  # 125440
N_PAD = N_PER_CORE * N_CORES      # 1003520 (input padded up to this)

# Pack bit pairs on-device (pair[k] = bits[k] + 2*bits[k+24], one extra
# DVE scalar_tensor_tensor per tile) to halve the output DMA bytes.
PACK_PAIRS = True
NUM_PAIRS = NUM_BITS // 2

_CACHE = {}


def _emit_body(tc, u_ap, p_ap, b_ap, schedule, repeat=1):
    """Emit the per-core program: for each tile (128 partitions x C samples
    x 48 bits), DMA u in, compare against the broadcast probs tile on DVE,
    DMA the uint8 bits out. Row i of the bits output corresponds to row i
    of u.

    repeat > 1 re-runs the whole pass (same input/output regions) — only
    used for benchmarking kernel duration via the t(repeat) slope."""
    import concourse.mybir as mybir
    from concourse.alu_op_type import AluOpType

    nc = tc.nc
    cmax = max(schedule)
    pack = PACK_PAIRS

    with (
        tc.tile_pool(name="pin", bufs=3) as pin,
        tc.tile_pool(name="pbits", bufs=3) as pbits,
        tc.tile_pool(name="pout", bufs=3) as pout,
        tc.tile_pool(name="pprobs", bufs=1) as pp,
    ):
        pt = pp.tile([128, NUM_BITS], mybir.dt.float32)
        nc.sync.dma_start(pt[:, :], p_ap)

        for _ in range(repeat):
            off = 0
            for c in schedule:
                uv = u_ap[off * 128:(off + c) * 128, :].rearrange(
                    "(p c) b -> p c b", p=128)
                bv = b_ap[off * 128:(off + c) * 128, :].rearrange(
                    "(p c) b -> p c b", p=128)

                ut = pin.tile([128, cmax * NUM_BITS], mybir.dt.float32,
                              tag="in")
                u3 = ut[:, :c * NUM_BITS].rearrange("p (c b) -> p c b",
                                                    b=NUM_BITS)
                nc.sync.dma_start(u3, uv)

                bt = pbits.tile([128, cmax * NUM_BITS], mybir.dt.uint8,
                                tag="bits")
                b3 = bt[:, :c * NUM_BITS].rearrange("p (c b) -> p c b",
                                                    b=NUM_BITS)
                pb = pt[:, :].unsqueeze(1).to_broadcast((128, c, NUM_BITS))
                nc.vector.tensor_tensor(b3, u3, pb, AluOpType.is_lt)

                if pack:
                    # pair[k] = bits[k] + 2*bits[k+24], k in [0, 24)
                    qt = pout.tile([128, cmax * NUM_PAIRS], mybir.dt.uint8,
                                   tag="out")
                    q3 = qt[:, :c * NUM_PAIRS].rearrange(
                        "p (c k) -> p c k", k=NUM_PAIRS)
                    lo = b3[:, :, 0:NUM_PAIRS]
                    hi = b3[:, :, NUM_PAIRS:NUM_BITS]
                    nc.vector.scalar_tensor_tensor(
                        q3, hi, 2.0, lo, AluOpType.mult, AluOpType.add)
                    nc.sync.dma_start(bv, q3)
                else:
                    nc.sync.dma_start(bv, b3)
                off += c


def _build(n_per_core, schedule, repeat=1):
    import concourse.bacc as bacc
    import concourse.mybir as mybir
    import concourse.tile as tile

    assert n_per_core == 128 * sum(schedule)
    nc = bacc.Bacc("TRN2", target_bir_lowering=False, debug=False)
    u = nc.dram_tensor("u", [n_per_core, NUM_BITS], mybir.dt.float32,
                       kind="ExternalInput")
    p = nc.dram_tensor("p", [128, NUM_BITS], mybir.dt.float32,
                       kind="ExternalInput")
    out_w = NUM_PAIRS if PACK_PAIRS else NUM_BITS
    b = nc.dram_tensor("bits", [n_per_core, out_w], mybir.dt.uint8,
                       kind="ExternalOutput")
    with tile.TileContext(nc) as tc:
        _emit_body(tc, u.ap(), p.ap()[:, :], b.ap(), schedule, repeat)
    nc.finalize()
    return nc


def _probs_f32(theta):
    """sigmoid(2*theta) computed exactly as the reference does, on the jax
    CPU backend (bit-exact vs a CPU-run reference)."""
    import jax

    cpu = jax.local_devices(backend="cpu")[0]
    theta_cpu = jax.device_put(np.asarray(theta, np.float32), cpu)
    with jax.default_device(cpu):
        probs = jax.nn.sigmoid(2.0 * theta_cpu)
    return np.asarray(probs, np.float32)


def _run_device(u_np, probs):
    from concourse.bass_utils import run_bass_kernel_spmd

    if "nc" not in _CACHE:
        _CACHE["nc"] = _build(N_PER_CORE, SCHEDULE)
    nc = _CACHE["nc"]

    # Pad with 2.0 (> any sigmoid output) => padding rows get all-zero bits.
    u_pad = np.empty((N_PAD, NUM_BITS), np.float32)
    u_pad[:N_TOTAL] = u_np
    u_pad[N_TOTAL:] = 2.0

    p_tile = np.broadcast_to(probs, (128, NUM_BITS)).copy()
    in_maps = [
        {"u": u_pad[k * N_PER_CORE:(k + 1) * N_PER_CORE], "p": p_tile}
        for k in range(N_CORES)
    ]
    res = run_bass_kernel_spmd(nc, in_maps, core_ids=list(range(N_CORES)))
    _CACHE["last_result"] = res
    out = np.concatenate([r["bits"] for r in res.results])[:N_TOTAL]
    if PACK_PAIRS:
        bits = np.empty((N_TOTAL, NUM_BITS), np.uint8)
        bits[:, :NUM_PAIRS] = out & 1
        bits[:, NUM_PAIRS:] = out >> 1
    else:
        bits = out
    return bits


def _dedup(bits_u8):
    """Replicate jnp.unique(keys, size=N, fill_value=-1) + scatter, where
    keys are the int32-overflowed packing (low 32 bits only) and duplicate
    scatter indices resolve last-write-wins (XLA CPU behavior)."""
    n = bits_u8.shape[0]
    packed = np.packbits(bits_u8, axis=1, bitorder="little")      # [n, 6]
    key32 = packed[:, :4].copy().view(np.uint32).ravel().view(np.int32)

    order = np.argsort(key32, kind="stable")
    sk = key32[order]
    mask = np.empty(n, bool)
    mask[0] = True
    mask[1:] = sk[1:] != sk[:-1]
    starts = np.nonzero(mask)[0]
    nuniq = len(starts)

    counts = np.zeros(n, np.int32)
    counts[:nuniq] = np.diff(np.append(starts, n)).astype(np.int32)

    winners = np.maximum.reduceat(order, starts)                  # last index
    unique_bits = np.zeros((n, NUM_BITS), np.int8)
    unique_bits[:nuniq] = bits_u8[winners]
    return unique_bits, counts


def kernel(theta, u):
    theta = np.asarray(theta, np.float32)
    u = np.ascontiguousarray(np.asarray(u, np.float32))
    assert u.shape == (N_TOTAL, NUM_BITS)

    probs = _probs_f32(theta)
    bits = _run_device(u, probs)
    return _dedup(bits)
